# revision 4
# baseline (speedup 1.0000x reference)
"""Gaussian-noise kernel for Trainium2: out = clip(x + noise, 0, 1).

Full input shape (64, 3, 512, 512) f32; pure data-parallel over the batch
dim across 8 NeuronCores (8 images per core). Per core the work is a flat
elementwise pass over 6,291,456 floats viewed as [128, 49152]: DMA x and
noise tiles into SBUF, add on the vector engine, clip with one dual-op
tensor_scalar (max 0, min 1), DMA the result back out.
"""

import numpy as np

import concourse.bacc as bacc
import concourse.bass as bass
import concourse.mybir as mybir
from concourse.bass_utils import run_bass_kernel_spmd
from concourse.tile import TileContext

N_CORES = 8
B, C, H, W = 64, 3, 512, 512
PER_CORE_ELEMS = (B // N_CORES) * C * H * W  # 6,291,456
P = 128
FREE = PER_CORE_ELEMS // P  # 49,152
CHUNK = 4096
N_CHUNKS = FREE // CHUNK  # 12

_cached_nc = None


def _build():
    nc = bacc.Bacc("TRN2", target_bir_lowering=False, debug=False)
    x = nc.dram_tensor("x", (P, FREE), mybir.dt.float32, kind="ExternalInput").ap()
    noise = nc.dram_tensor(
        "noise", (P, FREE), mybir.dt.float32, kind="ExternalInput"
    ).ap()
    out = nc.dram_tensor("out", (P, FREE), mybir.dt.float32, kind="ExternalOutput").ap()

    f32 = mybir.dt.float32
    with TileContext(nc) as tc:
        with tc.tile_pool(name="io", bufs=3) as pool:
            for i in range(N_CHUNKS):
                sl = bass.ts(i, CHUNK)
                xt = pool.tile([P, CHUNK], f32, tag="x")
                nt = pool.tile([P, CHUNK], f32, tag="n")
                nc.sync.dma_start(out=xt, in_=x[:, sl])
                nc.sync.dma_start(out=nt, in_=noise[:, sl])
                nc.vector.tensor_add(out=xt, in0=xt, in1=nt)
                nc.vector.tensor_scalar(
                    out=xt,
                    in0=xt,
                    scalar1=0.0,
                    scalar2=1.0,
                    op0=mybir.AluOpType.max,
                    op1=mybir.AluOpType.min,
                )
                nc.sync.dma_start(out=out[:, sl], in_=xt)
    nc.compile()
    return nc


def _get_nc():
    global _cached_nc
    if _cached_nc is None:
        _cached_nc = _build()
    return _cached_nc


def kernel(x: np.ndarray, noise: np.ndarray) -> np.ndarray:
    nc = _get_nc()
    xs = np.ascontiguousarray(x, dtype=np.float32).reshape(N_CORES, P, FREE)
    ns = np.ascontiguousarray(noise, dtype=np.float32).reshape(N_CORES, P, FREE)
    in_maps = [{"x": xs[c], "noise": ns[c]} for c in range(N_CORES)]
    res = run_bass_kernel_spmd(nc, in_maps, core_ids=list(range(N_CORES)))
    out = np.stack([res.results[c]["out"] for c in range(N_CORES)])
    return out.reshape(B, C, H, W)


# revision 5
# speedup vs baseline: 10.0643x; 10.0643x over previous
"""Gaussian-noise kernel for Trainium2: out = clip(x + noise, 0, 1).

Full input shape (64, 3, 512, 512) f32; pure data-parallel over the batch
dim across 8 NeuronCores (8 images per core). Per core the work is a flat
elementwise pass over 6,291,456 floats viewed as [128, 49152]: DMA x and
noise tiles into SBUF, add on the vector engine, clip with one dual-op
tensor_scalar (max 0, min 1), DMA the result back out.
"""

import numpy as np

import concourse.bacc as bacc
import concourse.bass as bass
import concourse.mybir as mybir
from concourse.bass_utils import run_bass_kernel_spmd
from concourse.tile import TileContext

N_CORES = 8
B, C, H, W = 64, 3, 512, 512
PER_CORE_ELEMS = (B // N_CORES) * C * H * W  # 6,291,456
P = 128
FREE = PER_CORE_ELEMS // P  # 49,152
CHUNK = 4096
N_CHUNKS = FREE // CHUNK  # 12

_cached_nc = None


def _build(repeat: int = 1):
    nc = bacc.Bacc("TRN2", target_bir_lowering=False, debug=False)
    x = nc.dram_tensor("x", (P, FREE), mybir.dt.float32, kind="ExternalInput").ap()
    noise = nc.dram_tensor(
        "noise", (P, FREE), mybir.dt.float32, kind="ExternalInput"
    ).ap()
    out = nc.dram_tensor("out", (P, FREE), mybir.dt.float32, kind="ExternalOutput").ap()

    f32 = mybir.dt.float32
    with TileContext(nc) as tc:
        with tc.tile_pool(name="io", bufs=3) as pool:

            def body():
                for i in range(N_CHUNKS):
                    sl = bass.ts(i, CHUNK)
                    xt = pool.tile([P, CHUNK], f32, tag="x")
                    nt = pool.tile([P, CHUNK], f32, tag="n")
                    nc.sync.dma_start(out=xt, in_=x[:, sl])
                    nc.sync.dma_start(out=nt, in_=noise[:, sl])
                    nc.vector.tensor_add(out=xt, in0=xt, in1=nt)
                    nc.vector.tensor_scalar(
                        out=xt,
                        in0=xt,
                        scalar1=0.0,
                        scalar2=1.0,
                        op0=mybir.AluOpType.max,
                        op1=mybir.AluOpType.min,
                    )
                    nc.sync.dma_start(out=out[:, sl], in_=xt)

            if repeat == 1:
                body()
            else:
                with tc.For_i(0, repeat, 1):
                    body()
    nc.compile()
    return nc


def _get_nc():
    global _cached_nc
    if _cached_nc is None:
        _cached_nc = _build()
    return _cached_nc


def kernel(x: np.ndarray, noise: np.ndarray) -> np.ndarray:
    nc = _get_nc()
    xs = np.ascontiguousarray(x, dtype=np.float32).reshape(N_CORES, P, FREE)
    ns = np.ascontiguousarray(noise, dtype=np.float32).reshape(N_CORES, P, FREE)
    in_maps = [{"x": xs[c], "noise": ns[c]} for c in range(N_CORES)]
    res = run_bass_kernel_spmd(nc, in_maps, core_ids=list(range(N_CORES)))
    out = np.stack([res.results[c]["out"] for c in range(N_CORES)])
    return out.reshape(B, C, H, W)


# revision 10
# speedup vs baseline: 10.3522x; 1.0286x over previous
"""Gaussian-noise kernel for Trainium2: out = clip(x + noise, 0, 1).

Full input shape (64, 3, 512, 512) f32; pure data-parallel over the batch
dim across 8 NeuronCores (8 images per core). Per core the work is a flat
elementwise pass over 6,291,456 floats: DMA x and noise tiles into SBUF,
add on the vector engine, clip with one dual-op tensor_scalar (max 0,
min 1), DMA the result back out.

The per-core flat buffer is viewed as [N_CHUNKS, 128, CHUNK] so each
chunk's DMA is one fully contiguous block of DRAM.
"""

import numpy as np

import concourse.bacc as bacc
import concourse.bass as bass
import concourse.mybir as mybir
from concourse.bass_utils import run_bass_kernel_spmd
from concourse.tile import TileContext

N_CORES = 8
B, C, H, W = 64, 3, 512, 512
PER_CORE_ELEMS = (B // N_CORES) * C * H * W  # 6,291,456
P = 128
FREE = PER_CORE_ELEMS // P  # 49,152

# tuned knobs
CHUNK = 4096
BUFS = 3
CONTIG = True          # view DRAM as [n_chunks, P, CHUNK] (contiguous chunks)
STORE_SCALAR = True    # issue store DMAs on the ACT HWDGE ring instead of SP
SPLIT_LOADS = True     # x loads on SP ring, noise loads on ACT ring
STORE_GPSIMD = False   # issue store DMAs via SWDGE (gpsimd) instead

_cached_nc = None


def _build(repeat: int = 1, chunk: int = CHUNK, bufs: int = BUFS,
           contig: bool = CONTIG, store_scalar: bool = STORE_SCALAR,
           split_loads: bool = SPLIT_LOADS, store_gpsimd: bool = STORE_GPSIMD):
    n_chunks = FREE // chunk
    assert n_chunks * chunk == FREE

    nc = bacc.Bacc("TRN2", target_bir_lowering=False, debug=False)
    f32 = mybir.dt.float32
    if contig:
        shape = (n_chunks, P, chunk)
    else:
        shape = (P, FREE)
    x = nc.dram_tensor("x", shape, f32, kind="ExternalInput").ap()
    noise = nc.dram_tensor("noise", shape, f32, kind="ExternalInput").ap()
    out = nc.dram_tensor("out", shape, f32, kind="ExternalOutput").ap()

    def chunk_ap(ap, i):
        if contig:
            return ap[i]
        return ap[:, bass.ts(i, chunk)]

    store_eng_load = nc.scalar if split_loads else nc.sync
    store_eng = nc.gpsimd if store_gpsimd else (nc.scalar if store_scalar else nc.sync)

    with TileContext(nc) as tc:
        with tc.tile_pool(name="io", bufs=bufs) as pool:

            def body():
                for i in range(n_chunks):
                    xt = pool.tile([P, chunk], f32, tag="x")
                    nt = pool.tile([P, chunk], f32, tag="n")
                    nc.sync.dma_start(out=xt, in_=chunk_ap(x, i))
                    store_eng_load.dma_start(out=nt, in_=chunk_ap(noise, i))
                    nc.vector.tensor_add(out=xt, in0=xt, in1=nt)
                    nc.vector.tensor_scalar(
                        out=xt,
                        in0=xt,
                        scalar1=0.0,
                        scalar2=1.0,
                        op0=mybir.AluOpType.max,
                        op1=mybir.AluOpType.min,
                    )
                    store_eng.dma_start(out=chunk_ap(out, i), in_=xt)

            if repeat == 1:
                body()
            else:
                with tc.For_i(0, repeat, 1):
                    body()
    nc.compile()
    return nc


def _get_nc():
    global _cached_nc
    if _cached_nc is None:
        _cached_nc = _build()
    return _cached_nc


def _shard(a: np.ndarray, contig: bool = CONTIG, chunk: int = CHUNK):
    n_chunks = FREE // chunk
    a = np.ascontiguousarray(a, dtype=np.float32)
    if contig:
        return a.reshape(N_CORES, n_chunks, P, chunk)
    return a.reshape(N_CORES, P, FREE)


# Cached PJRT executor: trace/compile the sharded bass_exec once per process
# so repeat kernel() calls only pay data transfer + execution.
_cached_fn = None


def _get_fn():
    global _cached_fn
    if _cached_fn is not None:
        return _cached_fn

    import jax
    from jax.sharding import Mesh, NamedSharding, PartitionSpec
    from jax.experimental.shard_map import shard_map
    from concourse.bass2jax import (
        _bass_exec_p,
        install_neuronx_cc_hook,
        partition_id_tensor,
    )

    nc = _get_nc()
    install_neuronx_cc_hook()
    partition_name = nc.partition_id_tensor.name if nc.partition_id_tensor else None

    in_names, out_names, out_avals, zero_outs = [], [], [], []
    for alloc in nc.m.functions[0].allocations:
        if not isinstance(alloc, mybir.MemoryLocationSet):
            continue
        name = alloc.memorylocations[0].name
        if alloc.kind == "ExternalInput":
            if name != partition_name:
                in_names.append(name)
        elif alloc.kind == "ExternalOutput":
            out_names.append(name)
            shape = tuple(alloc.tensor_shape)
            dtype = mybir.dt.np(alloc.dtype)
            out_avals.append(jax.core.ShapedArray(shape, dtype))
            zero_outs.append(np.zeros(shape, dtype))
    n_params = len(in_names)
    all_in_names = list(in_names) + list(out_names)
    if partition_name is not None:
        all_in_names.append(partition_name)

    def _body(*args):
        operands = list(args)
        if partition_name is not None:
            operands.append(partition_id_tensor())
        outs = _bass_exec_p.bind(
            *operands,
            out_avals=tuple(out_avals),
            in_names=tuple(all_in_names),
            out_names=tuple(out_names),
            lowering_input_output_aliases=(),
            sim_require_finite=True,
            sim_require_nnan=True,
            nc=nc,
        )
        return tuple(outs)

    devices = jax.devices()[:N_CORES]
    mesh = Mesh(np.asarray(devices), ("core",))
    in_specs = (PartitionSpec("core"),) * (n_params + len(out_names))
    out_specs = (PartitionSpec("core"),) * len(out_names)
    fn = jax.jit(
        shard_map(_body, mesh=mesh, in_specs=in_specs, out_specs=out_specs,
                  check_rep=False),
        keep_unused=True,
    )
    sharding = NamedSharding(mesh, PartitionSpec("core"))
    zeros_global = [np.concatenate([z] * N_CORES, axis=0) for z in zero_outs]
    _cached_fn = (fn, in_names, sharding, zeros_global)
    return _cached_fn


def _kernel_fast(x: np.ndarray, noise: np.ndarray) -> np.ndarray:
    import jax

    fn, in_names, sharding, zeros_global = _get_fn()
    per_core = {"x": _shard(x), "noise": _shard(noise)}
    args = []
    for name in in_names:
        a = per_core[name]
        args.append(jax.device_put(
            np.ascontiguousarray(a.reshape(-1, *a.shape[2:])), sharding))
    for z in zeros_global:
        args.append(jax.device_put(z, sharding))
    out = fn(*args)[0]
    return np.asarray(out).reshape(B, C, H, W)


def _kernel_stock(x: np.ndarray, noise: np.ndarray) -> np.ndarray:
    nc = _get_nc()
    xs = _shard(x)
    ns = _shard(noise)
    in_maps = [{"x": xs[c], "noise": ns[c]} for c in range(N_CORES)]
    res = run_bass_kernel_spmd(nc, in_maps, core_ids=list(range(N_CORES)))
    out = np.stack([res.results[c]["out"] for c in range(N_CORES)])
    return out.reshape(B, C, H, W)


_fast_broken = False


def kernel(x: np.ndarray, noise: np.ndarray) -> np.ndarray:
    global _fast_broken
    if not _fast_broken:
        try:
            return _kernel_fast(x, noise)
        except Exception:
            _fast_broken = True
    return _kernel_stock(x, noise)
